# revision 4
# baseline (speedup 1.0000x reference)
"""MultiHeadMlp TRN2 kernel: grouped per-head MLP + SE channel attention.

Full-input contract: kernel(**inputs) takes the complete arrays and returns
the complete output. Internally shards data-parallel over the batch dim
(B=8 -> 8 NeuronCores), builds one SPMD Bass/Tile program, and runs it via
run_bass_kernel_spmd.

Math (per batch element b, all tokens local to one core):
    xh = x.reshape(N, H, D)
    h  = gelu(xh @ W1 + b1)          per head, D=256 -> HID=1024
    o  = h @ W2 + b2                 per head, HID   -> D
    out = concat_heads(o)            (N, C)
    pooled = out.mean(axis=0)        (C,)
    gate = sigmoid(relu(pooled@cw1+cb1)@cw2+cb2)
    y = out * (1 + gate)

Layout strategy: activations run channel-major ("transposed") through both
GEMMs, so W1 [D,HID] and W2 [HID,D] serve directly as lhsT and the SE pool
is a free-dim reduction. x is loaded channel-major with a few large DMA
transposes (the xbar-efficient direction); o^T is transposed back to
token-major on the TensorEngine (identity matmul), which overlaps with the
GEMM stream. Only the gate-multiply + store trail the compute.
"""

import numpy as np
import ml_dtypes

B = 8
N = 4096
DIM = 1024
H = 4
HD = 256           # head dim
HID = 1024         # per-head hidden
SQ = 64            # squeeze dim
TCH = 512          # tokens per chunk
NCHUNK = N // TCH  # 8
NTOK_TILES = N // 128  # 32
NCORES = 8

_BF = ml_dtypes.bfloat16

_cache = {}


def _build():
    from contextlib import ExitStack

    import concourse.bass as bass
    import concourse.mybir as mybir
    from concourse import bacc
    from concourse.masks import make_identity
    from concourse.tile import TileContext

    dt = mybir.dt
    bf = dt.bfloat16
    f32 = dt.float32
    Act = mybir.ActivationFunctionType
    Alu = mybir.AluOpType
    Ax = mybir.AxisListType

    nc = bacc.Bacc("TRN2", target_bir_lowering=False, debug=False)

    x = nc.dram_tensor("x", [N, DIM], bf, kind="ExternalInput")
    w1 = nc.dram_tensor("w1", [H, HD, HID], bf, kind="ExternalInput")
    w2 = nc.dram_tensor("w2", [H, HID, HD], bf, kind="ExternalInput")
    b1t = nc.dram_tensor("b1t", [128, H * 8], f32, kind="ExternalInput")
    b2t = nc.dram_tensor("b2t", [128, 8], f32, kind="ExternalInput")
    cw1 = nc.dram_tensor("cw1", [DIM, SQ], bf, kind="ExternalInput")
    cb1t = nc.dram_tensor("cb1t", [SQ, 1], f32, kind="ExternalInput")
    cw2 = nc.dram_tensor("cw2", [SQ, DIM], bf, kind="ExternalInput")
    cb2 = nc.dram_tensor("cb2", [1, DIM], f32, kind="ExternalInput")
    out = nc.dram_tensor("out", [N, DIM], bf, kind="ExternalOutput")

    with TileContext(nc) as tc, ExitStack() as ctx:
        const = ctx.enter_context(tc.tile_pool(name="const", bufs=1))
        hpool = ctx.enter_context(tc.tile_pool(name="hpool", bufs=2))
        otpool = ctx.enter_context(tc.tile_pool(name="otpool", bufs=4))
        pg1 = ctx.enter_context(tc.tile_pool(name="pg1", bufs=2, space="PSUM"))
        pg2 = ctx.enter_context(tc.tile_pool(name="pg2", bufs=2, space="PSUM"))
        ptt = ctx.enter_context(tc.tile_pool(name="ptt", bufs=3, space="PSUM"))

        # ---- x, channel-major, loaded via large xbar transposes ----
        xfull = []
        for c in range(8):
            t = const.tile([128, N], bf, name=f"xfull_{c}", tag=f"xfull_{c}")
            for half in range(2):
                nc.sync.dma_start(
                    out=t[:, half * 2048:(half + 1) * 2048],
                    in_=x[half * 2048:(half + 1) * 2048,
                         c * 128:(c + 1) * 128],
                    transpose=True,
                )
            xfull.append(t)

        # ---- persistent weights / constants (SWDGE queues) ----
        w1sb = {}
        for h in range(H):
            for k in range(2):
                t = const.tile([128, HID], bf, name=f"w1sb_{h}_{k}",
                               tag=f"w1sb_{h}_{k}")
                nc.gpsimd.dma_start(out=t, in_=w1[h, k * 128:(k + 1) * 128, :])
                w1sb[h, k] = t
        w2sb = {}
        for h in range(H):
            for k in range(8):
                t = const.tile([128, HD], bf, name=f"w2sb_{h}_{k}",
                               tag=f"w2sb_{h}_{k}")
                nc.gpsimd.dma_start(out=t, in_=w2[h, k * 128:(k + 1) * 128, :])
                w2sb[h, k] = t
        b1sb = const.tile([128, H * 8], f32, name="b1sb", tag="b1sb")
        nc.gpsimd.dma_start(out=b1sb, in_=b1t[:, :])
        b2sb = const.tile([128, 8], f32, name="b2sb", tag="b2sb")
        nc.gpsimd.dma_start(out=b2sb, in_=b2t[:, :])
        cw1sb = {}
        for c in range(8):
            t = const.tile([128, SQ], bf, name=f"cw1sb_{c}", tag=f"cw1sb_{c}")
            nc.gpsimd.dma_start(out=t, in_=cw1[c * 128:(c + 1) * 128, :])
            cw1sb[c] = t
        cb1sb = const.tile([SQ, 1], f32, name="cb1sb", tag="cb1sb")
        nc.gpsimd.dma_start(out=cb1sb, in_=cb1t[:, :])
        cw2sb = const.tile([SQ, DIM], bf, name="cw2sb", tag="cw2sb")
        nc.gpsimd.dma_start(out=cw2sb, in_=cw2[:, :])
        cb2sb = const.tile([1, DIM], f32, name="cb2sb", tag="cb2sb")
        nc.gpsimd.dma_start(out=cb2sb, in_=cb2[:, :])
        ones1 = const.tile([1, 128], bf, name="ones1", tag="ones1")
        nc.vector.memset(ones1, 1.0)
        ident = const.tile([128, 128], bf, name="ident", tag="ident")
        make_identity(nc, ident)

        # token-major out accumulator (persists across whole kernel)
        otm = []
        for j in range(NTOK_TILES):
            t = const.tile([128, DIM], bf, name=f"otm_{j}", tag=f"otm_{j}")
            otm.append(t)
        # per-(chunk, chan-tile) row sums for the SE pool
        prow = const.tile([128, NCHUNK * 8], f32, name="prow", tag="prow")

        # ---- main loop over token chunks ----
        for i in range(NCHUNK):
            t0 = i * TCH
            for h in range(H):
                # GEMM1: h^T[m-tile] = gelu(W1_h^T x^T + b1)
                ht = []
                for m in range(8):
                    p1 = pg1.tile([128, TCH], f32, name="p1", tag="p1")
                    nc.tensor.matmul(
                        p1, lhsT=w1sb[h, 0][:, m * 128:(m + 1) * 128],
                        rhs=xfull[2 * h][:, t0:t0 + TCH],
                        start=True, stop=False)
                    nc.tensor.matmul(
                        p1, lhsT=w1sb[h, 1][:, m * 128:(m + 1) * 128],
                        rhs=xfull[2 * h + 1][:, t0:t0 + TCH],
                        start=False, stop=True)
                    hm = hpool.tile([128, TCH], bf, name=f"ht_{m}",
                                    tag=f"ht_{m}")
                    nc.scalar.activation(
                        out=hm, in_=p1, func=Act.Gelu,
                        bias=b1sb[:, h * 8 + m:h * 8 + m + 1])
                    ht.append(hm)
                # GEMM2: o^T[d-half] = W2_h^T h^T + b2
                for d in range(2):
                    c = h * 2 + d
                    p2 = pg2.tile([128, TCH], f32, name="p2", tag="p2")
                    for k in range(8):
                        nc.tensor.matmul(
                            p2, lhsT=w2sb[h, k][:, d * 128:(d + 1) * 128],
                            rhs=ht[k], start=(k == 0), stop=(k == 7))
                    ot = otpool.tile([128, TCH], bf, name="ot", tag="ot")
                    nc.vector.tensor_scalar(
                        out=ot, in0=p2, scalar1=b2sb[:, c:c + 1],
                        scalar2=0.0, op0=Alu.add, op1=Alu.add,
                        accum_out=prow[:, i * 8 + c:i * 8 + c + 1])
                    # transpose o^T back to token-major on the PE
                    for t in range(4):
                        pt = ptt.tile([128, 128], bf, name="pt", tag="pt")
                        nc.tensor.transpose(
                            pt, ot[:, t * 128:(t + 1) * 128], ident)
                        nc.vector.tensor_copy(
                            out=otm[i * 4 + t][:, c * 128:(c + 1) * 128],
                            in_=pt)

        # ---- SE channel attention on pooled means ----
        pooled_raw = const.tile([128, 8], f32, name="pooled_raw",
                                tag="pooled_raw")
        prow3 = prow.rearrange("p (i c) -> p i c", c=8)
        for c in range(8):
            nc.vector.tensor_reduce(
                out=pooled_raw[:, c:c + 1], in_=prow3[:, :, c],
                axis=Ax.X, op=Alu.add)
        pooledT = const.tile([128, 8], bf, name="pooledT", tag="pooledT")
        nc.vector.tensor_scalar_mul(pooledT, pooled_raw, 1.0 / N)

        pz = pg1.tile([SQ, 1], f32, name="pz", tag="p1")
        for c in range(8):
            nc.tensor.matmul(pz, lhsT=cw1sb[c], rhs=pooledT[:, c:c + 1],
                             start=(c == 0), stop=(c == 7))
        z1sb = const.tile([SQ, 1], bf, name="z1sb", tag="z1sb")
        nc.scalar.activation(out=z1sb, in_=pz, func=Act.Relu, bias=cb1sb)

        gsb = const.tile([1, DIM], f32, name="gsb", tag="gsb")
        g1sb = const.tile([1, DIM], bf, name="g1sb", tag="g1sb")
        for n in range(2):
            gp = pg2.tile([1, TCH], f32, name="gp", tag="p2")
            nc.tensor.matmul(gp, lhsT=z1sb,
                             rhs=cw2sb[:, n * 512:(n + 1) * 512],
                             start=True, stop=True)
            nc.vector.tensor_tensor(
                out=gsb[:, n * 512:(n + 1) * 512], in0=gp,
                in1=cb2sb[:, n * 512:(n + 1) * 512], op=Alu.add)
            nc.scalar.activation(
                out=gsb[:, n * 512:(n + 1) * 512],
                in_=gsb[:, n * 512:(n + 1) * 512], func=Act.Sigmoid)
            # 1 + gate, in bf16 for the broadcast matmul
            nc.vector.tensor_scalar_add(
                g1sb[:, n * 512:(n + 1) * 512],
                gsb[:, n * 512:(n + 1) * 512], 1.0)

        gb = const.tile([128, DIM], bf, name="gb", tag="gb")
        for n in range(2):
            bp = pg1.tile([128, TCH], f32, name="bp", tag="p1")
            nc.tensor.matmul(bp, lhsT=ones1,
                             rhs=g1sb[:, n * 512:(n + 1) * 512],
                             start=True, stop=True)
            nc.vector.tensor_copy(out=gb[:, n * 512:(n + 1) * 512], in_=bp)

        # ---- final scale + store (in-place on otm) ----
        for j in range(NTOK_TILES):
            nc.vector.tensor_tensor(out=otm[j], in0=otm[j], in1=gb,
                                    op=Alu.mult)
            nc.gpsimd.dma_start(out=out[j * 128:(j + 1) * 128, :], in_=otm[j])

    nc.compile()
    return nc


def _get_nc():
    if "nc" not in _cache:
        _cache["nc"] = _build()
    return _cache["nc"]


def kernel(x, W1, b1, W2, b2, cw1, cb1, cw2, cb2):
    from concourse.bass_utils import run_bass_kernel_spmd

    nc = _get_nc()

    xb = np.asarray(x, dtype=_BF)               # (B, N, DIM)
    w1b = np.asarray(W1, dtype=_BF)
    w2b = np.asarray(W2, dtype=_BF)
    cw1b = np.asarray(cw1, dtype=_BF)
    cw2b = np.asarray(cw2, dtype=_BF)
    b1tv = np.ascontiguousarray(
        np.asarray(b1, np.float32).reshape(H, 8, 128).transpose(2, 0, 1)
        .reshape(128, H * 8))
    b2tv = np.ascontiguousarray(
        np.asarray(b2, np.float32).reshape(H, 2, 128).transpose(2, 0, 1)
        .reshape(128, 8))
    cb1v = np.asarray(cb1, np.float32).reshape(SQ, 1)
    cb2v = np.asarray(cb2, np.float32).reshape(1, DIM)

    shared = {
        "w1": w1b, "w2": w2b, "b1t": b1tv, "b2t": b2tv,
        "cw1": cw1b, "cb1t": cb1v, "cw2": cw2b, "cb2": cb2v,
    }
    in_maps = [dict(shared, x=np.ascontiguousarray(xb[i]))
               for i in range(NCORES)]

    res = run_bass_kernel_spmd(nc, in_maps, core_ids=list(range(NCORES)))
    y = np.stack([res.results[i]["out"] for i in range(NCORES)], axis=0)
    return y.astype(np.float32)


# revision 7
# speedup vs baseline: 1.6638x; 1.6638x over previous
"""MultiHeadMlp TRN2 kernel: grouped per-head MLP + SE channel attention.

Full-input contract: kernel(**inputs) takes the complete arrays and returns
the complete output. Internally shards data-parallel over the batch dim
(B=8 -> 8 NeuronCores), builds one SPMD Bass/Tile program, and runs it via
run_bass_kernel_spmd.

Math (per batch element b, all tokens local to one core):
    xh = x.reshape(N, H, D)
    h  = gelu(xh @ W1 + b1)          per head, D=256 -> HID=1024
    o  = h @ W2 + b2                 per head, HID   -> D
    out = concat_heads(o)            (N, C)
    pooled = out.mean(axis=0)        (C,)
    gate = sigmoid(relu(pooled@cw1+cb1)@cw2+cb2)
    y = out * (1 + gate)

Layout strategy: everything on-chip is channel-major ("transposed"):
the host hands the kernel x^T (and un-transposes y^T on the way out), so
W1 [D,HID] / W2 [HID,D] serve directly as matmul lhsT operands, the SE
pool is a free-dim reduction, the gate is a native per-partition scalar
multiply, and the device never transposes anything.
"""

import numpy as np
import ml_dtypes

B = 8
N = 4096
DIM = 1024
H = 4
HD = 256           # head dim
HID = 1024         # per-head hidden
SQ = 64            # squeeze dim
TCH = 512          # tokens per chunk
NCHUNK = N // TCH  # 8
NCORES = 8

_BF = ml_dtypes.bfloat16

_cache = {}


def _build():
    from contextlib import ExitStack

    import concourse.bass as bass
    import concourse.mybir as mybir
    from concourse import bacc
    from concourse.tile import TileContext

    dt = mybir.dt
    bf = dt.bfloat16
    f32 = dt.float32
    Act = mybir.ActivationFunctionType
    Alu = mybir.AluOpType
    Ax = mybir.AxisListType

    nc = bacc.Bacc("TRN2", target_bir_lowering=False, debug=False)

    xt = nc.dram_tensor("xt", [DIM, N], bf, kind="ExternalInput")
    w1 = nc.dram_tensor("w1", [H, HD, HID], bf, kind="ExternalInput")
    w2 = nc.dram_tensor("w2", [H, HID, HD], bf, kind="ExternalInput")
    b1t = nc.dram_tensor("b1t", [128, H * 8], f32, kind="ExternalInput")
    b2t = nc.dram_tensor("b2t", [128, 8], f32, kind="ExternalInput")
    cw1 = nc.dram_tensor("cw1", [DIM, SQ], bf, kind="ExternalInput")
    cb1t = nc.dram_tensor("cb1t", [SQ, 1], f32, kind="ExternalInput")
    cw2 = nc.dram_tensor("cw2", [SQ, DIM], bf, kind="ExternalInput")
    cb2t = nc.dram_tensor("cb2t", [128, 8], f32, kind="ExternalInput")
    outT = nc.dram_tensor("outT", [DIM, N], bf, kind="ExternalOutput")

    with TileContext(nc) as tc, ExitStack() as ctx:
        const = ctx.enter_context(tc.tile_pool(name="const", bufs=1))
        hpool = ctx.enter_context(tc.tile_pool(name="hpool", bufs=2))
        pg1 = ctx.enter_context(tc.tile_pool(name="pg1", bufs=3, space="PSUM"))
        pg2 = ctx.enter_context(tc.tile_pool(name="pg2", bufs=3, space="PSUM"))

        # ---- x^T resident in SBUF, plain contiguous loads ----
        xfull = []
        for c in range(8):
            t = const.tile([128, N], bf, name=f"xfull_{c}", tag=f"xfull_{c}")
            nc.sync.dma_start(out=t, in_=xt[c * 128:(c + 1) * 128, :])
            xfull.append(t)

        # ---- persistent weights / constants ----
        w1sb = {}
        for h in range(H):
            for k in range(2):
                t = const.tile([128, HID], bf, name=f"w1sb_{h}_{k}",
                               tag=f"w1sb_{h}_{k}")
                nc.gpsimd.dma_start(out=t, in_=w1[h, k * 128:(k + 1) * 128, :])
                w1sb[h, k] = t
        w2sb = {}
        for h in range(H):
            for k in range(8):
                t = const.tile([128, HD], bf, name=f"w2sb_{h}_{k}",
                               tag=f"w2sb_{h}_{k}")
                nc.gpsimd.dma_start(out=t, in_=w2[h, k * 128:(k + 1) * 128, :])
                w2sb[h, k] = t
        b1sb = const.tile([128, H * 8], f32, name="b1sb", tag="b1sb")
        nc.gpsimd.dma_start(out=b1sb, in_=b1t[:, :])
        b2sb = const.tile([128, 8], f32, name="b2sb", tag="b2sb")
        nc.gpsimd.dma_start(out=b2sb, in_=b2t[:, :])
        cw1sb = {}
        for c in range(8):
            t = const.tile([128, SQ], bf, name=f"cw1sb_{c}", tag=f"cw1sb_{c}")
            nc.gpsimd.dma_start(out=t, in_=cw1[c * 128:(c + 1) * 128, :])
            cw1sb[c] = t
        cb1sb = const.tile([SQ, 1], f32, name="cb1sb", tag="cb1sb")
        nc.gpsimd.dma_start(out=cb1sb, in_=cb1t[:, :])
        cw2sb = const.tile([SQ, DIM], bf, name="cw2sb", tag="cw2sb")
        nc.gpsimd.dma_start(out=cw2sb, in_=cw2[:, :])
        cb2sb = const.tile([128, 8], f32, name="cb2sb", tag="cb2sb")
        nc.gpsimd.dma_start(out=cb2sb, in_=cb2t[:, :])

        # channel-major out accumulator (persists across whole kernel)
        oT = []
        for c in range(8):
            t = const.tile([128, N], bf, name=f"oT_{c}", tag=f"oT_{c}")
            oT.append(t)
        # per-(chunk, chan-tile) row sums for the SE pool
        prow = const.tile([128, NCHUNK * 8], f32, name="prow", tag="prow")

        # ---- main loop over token chunks ----
        for i in range(NCHUNK):
            t0 = i * TCH
            for h in range(H):
                # GEMM1: h^T[m-tile] = gelu(W1_h^T x^T + b1)
                ht = []
                for m in range(8):
                    p1 = pg1.tile([128, TCH], f32, name="p1", tag="p1")
                    nc.tensor.matmul(
                        p1, lhsT=w1sb[h, 0][:, m * 128:(m + 1) * 128],
                        rhs=xfull[2 * h][:, t0:t0 + TCH],
                        start=True, stop=False)
                    nc.tensor.matmul(
                        p1, lhsT=w1sb[h, 1][:, m * 128:(m + 1) * 128],
                        rhs=xfull[2 * h + 1][:, t0:t0 + TCH],
                        start=False, stop=True)
                    hm = hpool.tile([128, TCH], bf, name=f"ht_{m}",
                                    tag=f"ht_{m}")
                    nc.scalar.activation(
                        out=hm, in_=p1, func=Act.Gelu,
                        bias=b1sb[:, h * 8 + m:h * 8 + m + 1])
                    ht.append(hm)
                # GEMM2: o^T[d-half] = W2_h^T h^T + b2
                for d in range(2):
                    c = h * 2 + d
                    p2 = pg2.tile([128, TCH], f32, name="p2", tag="p2")
                    for k in range(8):
                        nc.tensor.matmul(
                            p2, lhsT=w2sb[h, k][:, d * 128:(d + 1) * 128],
                            rhs=ht[k], start=(k == 0), stop=(k == 7))
                    nc.vector.tensor_scalar(
                        out=oT[c][:, t0:t0 + TCH], in0=p2,
                        scalar1=b2sb[:, c:c + 1],
                        scalar2=0.0, op0=Alu.add, op1=Alu.add,
                        accum_out=prow[:, i * 8 + c:i * 8 + c + 1])

        # ---- SE channel attention on pooled means (all channel-major) ----
        pooled_raw = const.tile([128, 8], f32, name="pooled_raw",
                                tag="pooled_raw")
        prow3 = prow.rearrange("p (i c) -> p i c", c=8)
        for c in range(8):
            nc.vector.tensor_reduce(
                out=pooled_raw[:, c:c + 1], in_=prow3[:, :, c],
                axis=Ax.X, op=Alu.add)
        pooledT = const.tile([128, 8], bf, name="pooledT", tag="pooledT")
        nc.vector.tensor_scalar_mul(pooledT, pooled_raw, 1.0 / N)

        pz = pg1.tile([SQ, 1], f32, name="pz", tag="p1")
        for c in range(8):
            nc.tensor.matmul(pz, lhsT=cw1sb[c], rhs=pooledT[:, c:c + 1],
                             start=(c == 0), stop=(c == 7))
        z1sb = const.tile([SQ, 1], bf, name="z1sb", tag="z1sb")
        nc.scalar.activation(out=z1sb, in_=pz, func=Act.Relu, bias=cb1sb)

        # gate^T[c] = 1 + sigmoid(cw2^T z1 + cb2), per chan-tile
        g1T = const.tile([128, 8], f32, name="g1T", tag="g1T")
        for c in range(8):
            gp = pg2.tile([128, 1], f32, name="gp", tag="p2")
            nc.tensor.matmul(gp, lhsT=cw2sb[:, c * 128:(c + 1) * 128],
                             rhs=z1sb, start=True, stop=True)
            nc.scalar.activation(out=g1T[:, c:c + 1], in_=gp,
                                 func=Act.Sigmoid, bias=cb2sb[:, c:c + 1])
        nc.vector.tensor_scalar_add(g1T, g1T, 1.0)

        # ---- final scale + store (in-place on oT) ----
        for c in range(8):
            for half in range(2):
                sl = slice(half * 2048, (half + 1) * 2048)
                nc.vector.tensor_scalar_mul(
                    oT[c][:, sl], oT[c][:, sl], g1T[:, c:c + 1])
                nc.sync.dma_start(out=outT[c * 128:(c + 1) * 128, sl],
                                  in_=oT[c][:, sl])

    nc.compile()
    return nc


def _get_nc():
    if "nc" not in _cache:
        _cache["nc"] = _build()
    return _cache["nc"]


def _make_in_maps(x, W1, b1, W2, b2, cw1, cb1, cw2, cb2):
    # bf16 + pre-transposed x: (B, N, DIM) -> per-core (DIM, N)
    xb = np.asarray(x, dtype=_BF)
    w1b = np.asarray(W1, dtype=_BF)
    w2b = np.asarray(W2, dtype=_BF)
    cw1b = np.asarray(cw1, dtype=_BF)
    cw2b = np.asarray(cw2, dtype=_BF)
    b1tv = np.ascontiguousarray(
        np.asarray(b1, np.float32).reshape(H, 8, 128).transpose(2, 0, 1)
        .reshape(128, H * 8))
    b2tv = np.ascontiguousarray(
        np.asarray(b2, np.float32).reshape(H, 2, 128).transpose(2, 0, 1)
        .reshape(128, 8))
    cb1v = np.asarray(cb1, np.float32).reshape(SQ, 1)
    cb2tv = np.ascontiguousarray(
        np.asarray(cb2, np.float32).reshape(8, 128).T)

    shared = {
        "w1": w1b, "w2": w2b, "b1t": b1tv, "b2t": b2tv,
        "cw1": cw1b, "cb1t": cb1v, "cw2": cw2b, "cb2t": cb2tv,
    }
    return [dict(shared, xt=np.ascontiguousarray(xb[i].T))
            for i in range(NCORES)]


def kernel(x, W1, b1, W2, b2, cw1, cb1, cw2, cb2):
    from concourse.bass_utils import run_bass_kernel_spmd

    nc = _get_nc()
    in_maps = _make_in_maps(x, W1, b1, W2, b2, cw1, cb1, cw2, cb2)
    res = run_bass_kernel_spmd(nc, in_maps, core_ids=list(range(NCORES)))
    # un-transpose: per-core (DIM, N) -> (N, DIM)
    y = np.stack([res.results[i]["outT"].T for i in range(NCORES)], axis=0)
    return y.astype(np.float32)


# revision 11
# speedup vs baseline: 1.8206x; 1.0942x over previous
"""MultiHeadMlp TRN2 kernel: grouped per-head MLP + SE channel attention.

Full-input contract: kernel(**inputs) takes the complete arrays and returns
the complete output. Internally shards data-parallel over the batch dim
(B=8 -> 8 NeuronCores), builds one SPMD Bass/Tile program, and runs it via
run_bass_kernel_spmd.

Math (per batch element b, all tokens local to one core):
    xh = x.reshape(N, H, D)
    h  = gelu(xh @ W1 + b1)          per head, D=256 -> HID=1024
    o  = h @ W2 + b2                 per head, HID   -> D
    out = concat_heads(o)            (N, C)
    pooled = out.mean(axis=0)        (C,)
    gate = sigmoid(relu(pooled@cw1+cb1)@cw2+cb2)
    y = out * (1 + gate)

Layout strategy: everything on-chip is channel-major ("transposed"):
the host hands the kernel x^T (and un-transposes y^T on the way out), so
W1 [D,HID] / W2 [HID,D] serve directly as matmul lhsT operands, the SE
pool is a free-dim reduction, the gate is a native per-partition scalar
multiply, and the device never transposes anything.
"""

import numpy as np
import ml_dtypes

B = 8
N = 4096
DIM = 1024
H = 4
HD = 256           # head dim
HID = 1024         # per-head hidden
SQ = 64            # squeeze dim
TCH = 512          # tokens per chunk
NCHUNK = N // TCH  # 8
NCORES = 8

_BF = ml_dtypes.bfloat16

_cache = {}


def _build():
    from contextlib import ExitStack

    import concourse.bass as bass
    import concourse.mybir as mybir
    from concourse import bacc
    from concourse.tile import TileContext

    dt = mybir.dt
    bf = dt.bfloat16
    f32 = dt.float32
    Act = mybir.ActivationFunctionType
    Alu = mybir.AluOpType
    Ax = mybir.AxisListType

    nc = bacc.Bacc("TRN2", target_bir_lowering=False, debug=False)

    xt = nc.dram_tensor("xt", [DIM, N], bf, kind="ExternalInput")
    w1 = nc.dram_tensor("w1", [H, HD, HID], bf, kind="ExternalInput")
    w2 = nc.dram_tensor("w2", [H, HID, HD], bf, kind="ExternalInput")
    b1t = nc.dram_tensor("b1t", [128, H * 8], f32, kind="ExternalInput")
    b2t = nc.dram_tensor("b2t", [128, 8], f32, kind="ExternalInput")
    cw1 = nc.dram_tensor("cw1", [DIM, SQ], bf, kind="ExternalInput")
    cb1t = nc.dram_tensor("cb1t", [SQ, 1], f32, kind="ExternalInput")
    cw2 = nc.dram_tensor("cw2", [SQ, DIM], bf, kind="ExternalInput")
    cb2t = nc.dram_tensor("cb2t", [128, 8], f32, kind="ExternalInput")
    outT = nc.dram_tensor("outT", [DIM, N], bf, kind="ExternalOutput")

    with TileContext(nc) as tc, ExitStack() as ctx:
        const = ctx.enter_context(tc.tile_pool(name="const", bufs=1))
        hpool = ctx.enter_context(tc.tile_pool(name="hpool", bufs=2))
        pg1 = ctx.enter_context(tc.tile_pool(name="pg1", bufs=3, space="PSUM"))
        pg2 = ctx.enter_context(tc.tile_pool(name="pg2", bufs=3, space="PSUM"))

        # ---- activation-table warmup (overlaps the load phase) ----
        warm = const.tile([128, 1], f32, name="warm", tag="warm")
        nc.vector.memset(warm, 0.0)
        nc.scalar.activation(out=warm, in_=warm, func=Act.Sigmoid)
        nc.scalar.activation(out=warm, in_=warm, func=Act.Relu)
        nc.scalar.activation(out=warm, in_=warm, func=Act.Gelu)

        # ---- weights + x^T, ordered by first use, all on HWDGE ----
        # consolidated per-head weight tiles: one DMA each
        w1sb = [const.tile([128, 2, HID], bf, name=f"w1sb_{h}",
                           tag=f"w1sb_{h}") for h in range(H)]
        w2sb = [const.tile([128, 8, HD], bf, name=f"w2sb_{h}",
                           tag=f"w2sb_{h}") for h in range(H)]
        xfull = [const.tile([128, N], bf, name=f"xfull_{c}",
                            tag=f"xfull_{c}") for c in range(8)]
        b1sb = const.tile([128, H * 8], f32, name="b1sb", tag="b1sb")
        b2sb = const.tile([128, 8], f32, name="b2sb", tag="b2sb")

        nc.sync.dma_start(out=w1sb[0],
                          in_=w1[0].rearrange("(k p) n -> p k n", p=128))
        nc.sync.dma_start(out=b1sb, in_=b1t[:, :])
        nc.sync.dma_start(out=xfull[0], in_=xt[0:128, :])
        nc.sync.dma_start(out=xfull[1], in_=xt[128:256, :])
        nc.sync.dma_start(out=w2sb[0],
                          in_=w2[0].rearrange("(k p) n -> p k n", p=128))
        nc.sync.dma_start(out=b2sb, in_=b2t[:, :])
        for h in range(1, H):
            nc.sync.dma_start(out=w1sb[h],
                              in_=w1[h].rearrange("(k p) n -> p k n", p=128))
            nc.sync.dma_start(out=xfull[2 * h], in_=xt[h * 256:h * 256 + 128, :])
            nc.sync.dma_start(out=xfull[2 * h + 1],
                              in_=xt[h * 256 + 128:(h + 1) * 256, :])
            nc.sync.dma_start(out=w2sb[h],
                              in_=w2[h].rearrange("(k p) n -> p k n", p=128))
        cw1sb = const.tile([128, 8, SQ], bf, name="cw1sb", tag="cw1sb")
        nc.sync.dma_start(out=cw1sb,
                          in_=cw1.rearrange("(c p) n -> p c n", p=128))
        cb1sb = const.tile([SQ, 1], f32, name="cb1sb", tag="cb1sb")
        nc.sync.dma_start(out=cb1sb, in_=cb1t[:, :])
        cw2sb = const.tile([SQ, DIM], bf, name="cw2sb", tag="cw2sb")
        nc.sync.dma_start(out=cw2sb, in_=cw2[:, :])
        cb2sb = const.tile([128, 8], f32, name="cb2sb", tag="cb2sb")
        nc.sync.dma_start(out=cb2sb, in_=cb2t[:, :])

        # channel-major out accumulator (persists across whole kernel)
        oT = []
        for c in range(8):
            t = const.tile([128, N], bf, name=f"oT_{c}", tag=f"oT_{c}")
            oT.append(t)
        # per-(chunk, chan-tile) row sums for the SE pool
        prow = const.tile([128, NCHUNK * 8], f32, name="prow", tag="prow")

        # ---- main loop over token chunks ----
        for i in range(NCHUNK):
            t0 = i * TCH
            for h in range(H):
                # GEMM1: h^T[m-tile] = gelu(W1_h^T x^T + b1)
                ht = []
                for m in range(8):
                    p1 = pg1.tile([128, TCH], f32, name="p1", tag="p1")
                    nc.tensor.matmul(
                        p1, lhsT=w1sb[h][:, 0, m * 128:(m + 1) * 128],
                        rhs=xfull[2 * h][:, t0:t0 + TCH],
                        start=True, stop=False)
                    nc.tensor.matmul(
                        p1, lhsT=w1sb[h][:, 1, m * 128:(m + 1) * 128],
                        rhs=xfull[2 * h + 1][:, t0:t0 + TCH],
                        start=False, stop=True)
                    hm = hpool.tile([128, TCH], bf, name=f"ht_{m}",
                                    tag=f"ht_{m}")
                    nc.scalar.activation(
                        out=hm, in_=p1, func=Act.Gelu,
                        bias=b1sb[:, h * 8 + m:h * 8 + m + 1])
                    ht.append(hm)
                # GEMM2: o^T[d-half] = W2_h^T h^T + b2
                for d in range(2):
                    c = h * 2 + d
                    p2 = pg2.tile([128, TCH], f32, name="p2", tag="p2")
                    for k in range(8):
                        nc.tensor.matmul(
                            p2, lhsT=w2sb[h][:, k, d * 128:(d + 1) * 128],
                            rhs=ht[k], start=(k == 0), stop=(k == 7))
                    nc.vector.tensor_scalar(
                        out=oT[c][:, t0:t0 + TCH], in0=p2,
                        scalar1=b2sb[:, c:c + 1],
                        scalar2=0.0, op0=Alu.add, op1=Alu.add,
                        accum_out=prow[:, i * 8 + c:i * 8 + c + 1])

        # ---- SE channel attention on pooled means (all channel-major) ----
        pooled_raw = const.tile([128, 8], f32, name="pooled_raw",
                                tag="pooled_raw")
        prow3 = prow.rearrange("p (i c) -> p i c", c=8)
        for c in range(8):
            nc.vector.tensor_reduce(
                out=pooled_raw[:, c:c + 1], in_=prow3[:, :, c],
                axis=Ax.X, op=Alu.add)
        pooledT = const.tile([128, 8], bf, name="pooledT", tag="pooledT")
        nc.vector.tensor_scalar_mul(pooledT, pooled_raw, 1.0 / N)

        pz = pg1.tile([SQ, 1], f32, name="pz", tag="p1")
        for c in range(8):
            nc.tensor.matmul(pz, lhsT=cw1sb[:, c, :], rhs=pooledT[:, c:c + 1],
                             start=(c == 0), stop=(c == 7))
        z1sb = const.tile([SQ, 1], bf, name="z1sb", tag="z1sb")
        nc.scalar.activation(out=z1sb, in_=pz, func=Act.Relu, bias=cb1sb)

        # gate^T[c] = 1 + sigmoid(cw2^T z1 + cb2), per chan-tile
        g1T = const.tile([128, 8], f32, name="g1T", tag="g1T")
        for c in range(8):
            gp = pg2.tile([128, 1], f32, name="gp", tag="p2")
            nc.tensor.matmul(gp, lhsT=cw2sb[:, c * 128:(c + 1) * 128],
                             rhs=z1sb, start=True, stop=True)
            nc.scalar.activation(out=g1T[:, c:c + 1], in_=gp,
                                 func=Act.Sigmoid, bias=cb2sb[:, c:c + 1])
        nc.vector.tensor_scalar_add(g1T, g1T, 1.0)

        # ---- final scale + store (in-place on oT) ----
        for c in range(8):
            for half in range(2):
                sl = slice(half * 2048, (half + 1) * 2048)
                nc.vector.tensor_scalar_mul(
                    oT[c][:, sl], oT[c][:, sl], g1T[:, c:c + 1])
                nc.sync.dma_start(out=outT[c * 128:(c + 1) * 128, sl],
                                  in_=oT[c][:, sl])

    nc.compile()
    return nc


def _get_nc():
    if "nc" not in _cache:
        _cache["nc"] = _build()
    return _cache["nc"]


def _make_in_maps(x, W1, b1, W2, b2, cw1, cb1, cw2, cb2):
    # bf16 + pre-transposed x: (B, N, DIM) -> per-core (DIM, N)
    xb = np.asarray(x, dtype=_BF)
    w1b = np.asarray(W1, dtype=_BF)
    w2b = np.asarray(W2, dtype=_BF)
    cw1b = np.asarray(cw1, dtype=_BF)
    cw2b = np.asarray(cw2, dtype=_BF)
    b1tv = np.ascontiguousarray(
        np.asarray(b1, np.float32).reshape(H, 8, 128).transpose(2, 0, 1)
        .reshape(128, H * 8))
    b2tv = np.ascontiguousarray(
        np.asarray(b2, np.float32).reshape(H, 2, 128).transpose(2, 0, 1)
        .reshape(128, 8))
    cb1v = np.asarray(cb1, np.float32).reshape(SQ, 1)
    cb2tv = np.ascontiguousarray(
        np.asarray(cb2, np.float32).reshape(8, 128).T)

    shared = {
        "w1": w1b, "w2": w2b, "b1t": b1tv, "b2t": b2tv,
        "cw1": cw1b, "cb1t": cb1v, "cw2": cw2b, "cb2t": cb2tv,
    }
    return [dict(shared, xt=np.ascontiguousarray(xb[i].T))
            for i in range(NCORES)]


def kernel(x, W1, b1, W2, b2, cw1, cb1, cw2, cb2):
    from concourse.bass_utils import run_bass_kernel_spmd

    nc = _get_nc()
    in_maps = _make_in_maps(x, W1, b1, W2, b2, cw1, cb1, cw2, cb2)
    res = run_bass_kernel_spmd(nc, in_maps, core_ids=list(range(NCORES)))
    # un-transpose: per-core (DIM, N) -> (N, DIM)
    y = np.stack([res.results[i]["outT"].T for i in range(NCORES)], axis=0)
    return y.astype(np.float32)


# revision 14
# speedup vs baseline: 1.8995x; 1.0433x over previous
"""MultiHeadMlp TRN2 kernel: grouped per-head MLP + SE channel attention.

Full-input contract: kernel(**inputs) takes the complete arrays and returns
the complete output. Internally shards data-parallel over the batch dim
(B=8 -> 8 NeuronCores), builds one SPMD Bass/Tile program, and runs it via
run_bass_kernel_spmd.

Math (per batch element b, all tokens local to one core):
    xh = x.reshape(N, H, D)
    h  = gelu(xh @ W1 + b1)          per head, D=256 -> HID=1024
    o  = h @ W2 + b2                 per head, HID   -> D
    out = concat_heads(o)            (N, C)
    pooled = out.mean(axis=0)        (C,)
    gate = sigmoid(relu(pooled@cw1+cb1)@cw2+cb2)
    y = out * (1 + gate)

Layout strategy: everything on-chip is channel-major ("transposed"):
the host hands the kernel x^T (and un-transposes y^T on the way out), so
W1 [D,HID] / W2 [HID,D] serve directly as matmul lhsT operands, the SE
pool is a free-dim reduction, the gate is a native per-partition scalar
multiply, and the device never transposes anything.
"""

import numpy as np
import ml_dtypes

B = 8
N = 4096
DIM = 1024
H = 4
HD = 256           # head dim
HID = 1024         # per-head hidden
SQ = 64            # squeeze dim
TCH = 512          # tokens per chunk
NCHUNK = N // TCH  # 8
NCORES = 8

_BF = ml_dtypes.bfloat16

_cache = {}


def _build():
    from contextlib import ExitStack

    import concourse.bass as bass
    import concourse.mybir as mybir
    from concourse import bacc
    from concourse.tile import TileContext

    dt = mybir.dt
    bf = dt.bfloat16
    f32 = dt.float32
    Act = mybir.ActivationFunctionType
    Alu = mybir.AluOpType
    Ax = mybir.AxisListType

    nc = bacc.Bacc("TRN2", target_bir_lowering=False, debug=False)

    xt = nc.dram_tensor("xt", [DIM, N], bf, kind="ExternalInput")
    w1 = nc.dram_tensor("w1", [H, HD, HID], bf, kind="ExternalInput")
    w2 = nc.dram_tensor("w2", [H, HID, HD], bf, kind="ExternalInput")
    b1t = nc.dram_tensor("b1t", [128, H * 8], f32, kind="ExternalInput")
    b2t = nc.dram_tensor("b2t", [128, 8], f32, kind="ExternalInput")
    cw1 = nc.dram_tensor("cw1", [DIM, SQ], bf, kind="ExternalInput")
    cb1t = nc.dram_tensor("cb1t", [SQ, 1], f32, kind="ExternalInput")
    cw2 = nc.dram_tensor("cw2", [SQ, DIM], bf, kind="ExternalInput")
    cb2t = nc.dram_tensor("cb2t", [128, 8], f32, kind="ExternalInput")
    outT = nc.dram_tensor("outT", [DIM, N], bf, kind="ExternalOutput")

    with TileContext(nc) as tc, ExitStack() as ctx:
        const = ctx.enter_context(tc.tile_pool(name="const", bufs=1))
        hpool = ctx.enter_context(tc.tile_pool(name="hpool", bufs=2))
        pg1 = ctx.enter_context(tc.tile_pool(name="pg1", bufs=3, space="PSUM"))
        pg2 = ctx.enter_context(tc.tile_pool(name="pg2", bufs=3, space="PSUM"))

        # ---- activation-table warmup (overlaps the load phase) ----
        warm = const.tile([128, 1], f32, name="warm", tag="warm")
        nc.vector.memset(warm, 0.0)
        nc.scalar.activation(out=warm, in_=warm, func=Act.Sigmoid)
        nc.scalar.activation(out=warm, in_=warm, func=Act.Relu)
        nc.scalar.activation(out=warm, in_=warm, func=Act.Gelu)

        # ---- weights + x^T, ordered by first use, all on HWDGE ----
        # consolidated per-head weight tiles: one DMA each
        w1sb = [const.tile([128, 2, HID], bf, name=f"w1sb_{h}",
                           tag=f"w1sb_{h}") for h in range(H)]
        w2sb = [const.tile([128, 8, HD], bf, name=f"w2sb_{h}",
                           tag=f"w2sb_{h}") for h in range(H)]
        xfull = [const.tile([128, N], bf, name=f"xfull_{c}",
                            tag=f"xfull_{c}") for c in range(8)]
        b1sb = const.tile([128, H * 8], f32, name="b1sb", tag="b1sb")
        b2sb = const.tile([128, 8], f32, name="b2sb", tag="b2sb")

        HN = N // 2
        nc.sync.dma_start(out=w1sb[0],
                          in_=w1[0].rearrange("(k p) n -> p k n", p=128))
        nc.sync.dma_start(out=b1sb, in_=b1t[:, :])
        nc.sync.dma_start(out=xfull[0][:, :HN], in_=xt[0:128, :HN])
        nc.sync.dma_start(out=xfull[1][:, :HN], in_=xt[128:256, :HN])
        nc.sync.dma_start(out=w2sb[0],
                          in_=w2[0].rearrange("(k p) n -> p k n", p=128))
        nc.sync.dma_start(out=b2sb, in_=b2t[:, :])
        for h in range(1, H):
            nc.sync.dma_start(out=w1sb[h],
                              in_=w1[h].rearrange("(k p) n -> p k n", p=128))
            nc.sync.dma_start(out=xfull[2 * h][:, :HN],
                              in_=xt[h * 256:h * 256 + 128, :HN])
            nc.sync.dma_start(out=xfull[2 * h + 1][:, :HN],
                              in_=xt[h * 256 + 128:(h + 1) * 256, :HN])
            nc.sync.dma_start(out=w2sb[h],
                              in_=w2[h].rearrange("(k p) n -> p k n", p=128))
        for c in range(8):
            nc.sync.dma_start(out=xfull[c][:, HN:],
                              in_=xt[c * 128:(c + 1) * 128, HN:])
        cw1sb = const.tile([128, 8, SQ], bf, name="cw1sb", tag="cw1sb")
        nc.sync.dma_start(out=cw1sb,
                          in_=cw1.rearrange("(c p) n -> p c n", p=128))
        cb1sb = const.tile([SQ, 1], f32, name="cb1sb", tag="cb1sb")
        nc.sync.dma_start(out=cb1sb, in_=cb1t[:, :])
        cw2sb = const.tile([SQ, DIM], bf, name="cw2sb", tag="cw2sb")
        nc.sync.dma_start(out=cw2sb, in_=cw2[:, :])
        cb2sb = const.tile([128, 8], f32, name="cb2sb", tag="cb2sb")
        nc.sync.dma_start(out=cb2sb, in_=cb2t[:, :])

        # channel-major out accumulator (persists across whole kernel)
        oT = []
        for c in range(8):
            t = const.tile([128, N], bf, name=f"oT_{c}", tag=f"oT_{c}")
            oT.append(t)
        # per-(chunk, chan-tile) row sums for the SE pool
        prow = const.tile([128, NCHUNK * 8], f32, name="prow", tag="prow")

        # ---- main loop over token chunks ----
        for i in range(NCHUNK):
            t0 = i * TCH
            for h in range(H):
                # GEMM1: h^T[m-tile] = gelu(W1_h^T x^T + b1)
                ht = []
                for m in range(8):
                    p1 = pg1.tile([128, TCH], f32, name="p1", tag="p1")
                    nc.tensor.matmul(
                        p1, lhsT=w1sb[h][:, 0, m * 128:(m + 1) * 128],
                        rhs=xfull[2 * h][:, t0:t0 + TCH],
                        start=True, stop=False)
                    nc.tensor.matmul(
                        p1, lhsT=w1sb[h][:, 1, m * 128:(m + 1) * 128],
                        rhs=xfull[2 * h + 1][:, t0:t0 + TCH],
                        start=False, stop=True)
                    hm = hpool.tile([128, TCH], bf, name=f"ht_{m}",
                                    tag=f"ht_{m}")
                    nc.scalar.activation(
                        out=hm, in_=p1, func=Act.Gelu,
                        bias=b1sb[:, h * 8 + m:h * 8 + m + 1])
                    ht.append(hm)
                # GEMM2: o^T[d-half] = W2_h^T h^T + b2
                for d in range(2):
                    c = h * 2 + d
                    p2 = pg2.tile([128, TCH], f32, name="p2", tag="p2")
                    for k in range(8):
                        nc.tensor.matmul(
                            p2, lhsT=w2sb[h][:, k, d * 128:(d + 1) * 128],
                            rhs=ht[k], start=(k == 0), stop=(k == 7))
                    nc.vector.tensor_scalar(
                        out=oT[c][:, t0:t0 + TCH], in0=p2,
                        scalar1=b2sb[:, c:c + 1],
                        scalar2=0.0, op0=Alu.add, op1=Alu.add,
                        accum_out=prow[:, i * 8 + c:i * 8 + c + 1])

        # ---- SE channel attention on pooled means (all channel-major) ----
        # partial reduction over chunks 0..6 runs as soon as those chunks'
        # row sums exist (overlaps chunk 7 compute); only the final add is
        # on the critical path.
        pooled_part = const.tile([128, 8], f32, name="pooled_part",
                                 tag="pooled_part")
        pooled_raw = const.tile([128, 8], f32, name="pooled_raw",
                                tag="pooled_raw")
        prow3 = prow.rearrange("p (i c) -> p i c", c=8)
        for c in range(8):
            nc.vector.tensor_reduce(
                out=pooled_part[:, c:c + 1], in_=prow3[:, 0:NCHUNK - 1, c],
                axis=Ax.X, op=Alu.add)
        nc.vector.tensor_tensor(out=pooled_raw, in0=pooled_part,
                                in1=prow3[:, NCHUNK - 1, :], op=Alu.add)
        pooledT = const.tile([128, 8], bf, name="pooledT", tag="pooledT")
        nc.vector.tensor_scalar_mul(pooledT, pooled_raw, 1.0 / N)

        pz = pg1.tile([SQ, 1], f32, name="pz", tag="p1")
        for c in range(8):
            nc.tensor.matmul(pz, lhsT=cw1sb[:, c, :], rhs=pooledT[:, c:c + 1],
                             start=(c == 0), stop=(c == 7))
        z1sb = const.tile([SQ, 1], bf, name="z1sb", tag="z1sb")
        nc.scalar.activation(out=z1sb, in_=pz, func=Act.Relu, bias=cb1sb)

        # gate^T[c] = 1 + sigmoid(cw2^T z1 + cb2), per chan-tile
        g1T = const.tile([128, 8], f32, name="g1T", tag="g1T")
        for c in range(8):
            gp = pg2.tile([128, 1], f32, name="gp", tag="p2")
            nc.tensor.matmul(gp, lhsT=cw2sb[:, c * 128:(c + 1) * 128],
                             rhs=z1sb, start=True, stop=True)
            nc.scalar.activation(out=g1T[:, c:c + 1], in_=gp,
                                 func=Act.Sigmoid, bias=cb2sb[:, c:c + 1])
        nc.vector.tensor_scalar_add(g1T, g1T, 1.0)

        # ---- final scale + store (in-place on oT, split DVE/ACT) ----
        for c in range(8):
            for half in range(2):
                sl = slice(half * 2048, (half + 1) * 2048)
                if (c + half) % 2 == 0:
                    nc.vector.tensor_scalar_mul(
                        oT[c][:, sl], oT[c][:, sl], g1T[:, c:c + 1])
                else:
                    nc.scalar.activation(
                        out=oT[c][:, sl], in_=oT[c][:, sl],
                        func=Act.Copy, scale=g1T[:, c:c + 1])
                nc.sync.dma_start(out=outT[c * 128:(c + 1) * 128, sl],
                                  in_=oT[c][:, sl])

    nc.compile()
    return nc


def _get_nc():
    if "nc" not in _cache:
        _cache["nc"] = _build()
    return _cache["nc"]


def _make_in_maps(x, W1, b1, W2, b2, cw1, cb1, cw2, cb2):
    # bf16 + pre-transposed x: (B, N, DIM) -> per-core (DIM, N)
    xb = np.asarray(x, dtype=_BF)
    w1b = np.asarray(W1, dtype=_BF)
    w2b = np.asarray(W2, dtype=_BF)
    cw1b = np.asarray(cw1, dtype=_BF)
    cw2b = np.asarray(cw2, dtype=_BF)
    b1tv = np.ascontiguousarray(
        np.asarray(b1, np.float32).reshape(H, 8, 128).transpose(2, 0, 1)
        .reshape(128, H * 8))
    b2tv = np.ascontiguousarray(
        np.asarray(b2, np.float32).reshape(H, 2, 128).transpose(2, 0, 1)
        .reshape(128, 8))
    cb1v = np.asarray(cb1, np.float32).reshape(SQ, 1)
    cb2tv = np.ascontiguousarray(
        np.asarray(cb2, np.float32).reshape(8, 128).T)

    shared = {
        "w1": w1b, "w2": w2b, "b1t": b1tv, "b2t": b2tv,
        "cw1": cw1b, "cb1t": cb1v, "cw2": cw2b, "cb2t": cb2tv,
    }
    return [dict(shared, xt=np.ascontiguousarray(xb[i].T))
            for i in range(NCORES)]


def kernel(x, W1, b1, W2, b2, cw1, cb1, cw2, cb2):
    from concourse.bass_utils import run_bass_kernel_spmd

    nc = _get_nc()
    in_maps = _make_in_maps(x, W1, b1, W2, b2, cw1, cb1, cw2, cb2)
    res = run_bass_kernel_spmd(nc, in_maps, core_ids=list(range(NCORES)))
    # un-transpose: per-core (DIM, N) -> (N, DIM)
    y = np.stack([res.results[i]["outT"].T for i in range(NCORES)], axis=0)
    return y.astype(np.float32)
